# revision 16
# baseline (speedup 1.0000x reference)
"""DCNv2 (nn_DCNv2_63462436765991) Trainium2 Bass kernel.

Data-parallel over batch across 8 NeuronCores (2048 rows/core), 2
passes of 1024 rows.  Activations feature-major in SBUF (x^T), weights
stream from HBM as stationary matmul operands, fp32 PSUM accumulation.

Cross network AND MLP tower run in fp8e4m3 with DoubleRow perf mode
(2 fp8 weights per PE cell, 256-row contraction per matmul): x^T and
h^T live in double-k-tiles [128(q), 2(i), batch] with k = t*256 +
i*128 + q, and the pre-arranged weight blocks pair [q, i] identically
(verified against CoreSim).  Numerically validated end-to-end in
numpy: max rel err 3.2e-3 predicted vs the 2e-2 gate (3.3e-3 measured
on hardware).  x0 stays bf16 for the cross elementwise term.

All batch-independent tensors (embedding tables, weights) are baked
into the NEFF as Const DRAM tensors and land in HBM at model load;
per-call ExternalInputs are only the per-batch index/numeric data
(~1 MB/core).  Biases are folded as an extra k-row paired with a
ones-row in x; k-tiles are zero-padded to uniform loops.

Embedding gathers:
  - categorical: dma_gather(transpose=True) over host-padded bf16
    tables writes gathered rows feature-major directly into x^T.
  - user/item (vocab 100k > int16): indirect_dma_start one row per
    partition, then PE transpose into x^T.

x0 row layout: rows 0:64 user, 64:128 item, 128:960 numeric,
960:2624 categorical; row 2624 = ones (bias fold), rest zero pad.
"""

import hashlib
import numpy as np

B = 16384
CORES = 8
B_CORE = B // CORES            # 2048
N_PASS = 2
BC = B_CORE // N_PASS          # 1024 batch per pass
NCH = BC // 512                # matmul N-chunks per pass
EMB = 64
N_NUM = 13
N_CAT = 26
CAT_VOCAB = 10000
D = 2624
KT = 21                        # bf16 k-tiles over D (20 x 128 + 64 + bias)
DT = 11                        # fp8 double-k-tiles (22 x 128 padded)
MLP = 1024
MT = MLP // 128                # 8
L_CROSS = 4
N_MLP_HID = 3

_CACHE = {}


def _build_nc(shared, n_cross=L_CROSS, with_mlp=True,
              parts=("cat", "num", "uit"), repeats=1, wbufs=3, psbufs=3):
    import concourse.bass as bass
    import concourse.mybir as mybir
    import concourse.tile as tile
    from concourse import bacc
    from concourse.masks import make_identity

    f32 = mybir.dt.float32
    bf16 = mybir.dt.bfloat16
    f8 = mybir.dt.float8e4
    i32 = mybir.dt.int32
    i16 = mybir.dt.int16
    MULT = mybir.AluOpType.mult
    ADD = mybir.AluOpType.add
    RELU = mybir.ActivationFunctionType.Relu
    COPY = mybir.ActivationFunctionType.Copy
    SIGM = mybir.ActivationFunctionType.Sigmoid
    DR = mybir.MatmulPerfMode.DoubleRow

    nc = bacc.Bacc("TRN2", target_bir_lowering=False, debug=False)

    # ---- per-batch DRAM inputs ----
    u_idx_d = nc.dram_tensor("u_idx", [128, 16], i32, kind="ExternalInput")
    i_idx_d = nc.dram_tensor("i_idx", [128, 16], i32, kind="ExternalInput")
    c_idx_d = nc.dram_tensor("c_idx", [128, N_CAT * 128], i16, kind="ExternalInput")
    numT_d = nc.dram_tensor("numT", [N_NUM + 1, B_CORE], bf16, kind="ExternalInput")
    out_d = nc.dram_tensor("out", [1, B_CORE], f32, kind="ExternalOutput")

    # ---- NEFF-baked constants (land in HBM at model load) ----
    ndiag_d = nc.inline_tensor(shared["ndiag"], name="ndiag")
    uemb_d = nc.inline_tensor(shared["user_emb"], name="user_emb")
    iemb_d = nc.inline_tensor(shared["item_emb"], name="item_emb")
    cpad_d = nc.inline_tensor(shared["cat_pad"], name="cat_pad")
    Wc8_d = nc.inline_tensor(shared["WcP8"], name="WcP8")    # [4, 21, 128, 2816] f8
    W08_d = nc.inline_tensor(shared["W0P8"], name="W0P8")    # [8, 128, 2816] f8
    Wh8_d = nc.inline_tensor(shared["WhP8"], name="WhP8")    # [3, 8, 128, 1024] f8
    bhT_d = nc.inline_tensor(shared["bhT"], name="bhT")      # [1024, 3] f32
    Wf8_d = nc.inline_tensor(shared["WfP8x"], name="WfP8x")  # [128, 352] f8
    Wfh8_d = nc.inline_tensor(shared["WfPh8"], name="WfPh8") # [128, 128] f8

    with tile.TileContext(nc) as tc:
        from contextlib import ExitStack
        with ExitStack() as ctx:
            const = ctx.enter_context(tc.tile_pool(name="const", bufs=1))
            xpool = ctx.enter_context(tc.tile_pool(name="xpool", bufs=1))
            wpool = ctx.enter_context(tc.tile_pool(name="wpool", bufs=wbufs))
            stpool = ctx.enter_context(tc.tile_pool(name="stpool", bufs=2))
            tpool = ctx.enter_context(tc.tile_pool(name="tpool", bufs=4))
            bpool = ctx.enter_context(tc.tile_pool(name="bpool", bufs=2))
            zpool = ctx.enter_context(tc.tile_pool(name="zpool", bufs=2))
            mmps = ctx.enter_context(tc.tile_pool(name="mmps", bufs=psbufs, space="PSUM"))
            trps = ctx.enter_context(tc.tile_pool(name="trps", bufs=2, space="PSUM"))

            # ---- per-core constants ----
            uidx = const.tile([128, 16], i32)
            iidx = const.tile([128, 16], i32)
            cidx = const.tile([128, N_CAT * 128], i16)
            numT = const.tile([N_NUM + 1, B_CORE], bf16)
            ndiag = const.tile([N_NUM + 1, N_NUM * EMB], bf16)
            ident = const.tile([128, 128], f32)
            nc.sync.dma_start(uidx[:], u_idx_d[:])
            nc.sync.dma_start(iidx[:], i_idx_d[:])
            nc.sync.dma_start(cidx[:], c_idx_d[:])
            nc.sync.dma_start(numT[:], numT_d[:])
            nc.sync.dma_start(ndiag[:], ndiag_d[:])
            make_identity(nc, ident)

            def alloc_x8(prefix):
                return [xpool.tile([128, 2, BC], f8, tag=f"{prefix}{t}",
                                   name=f"{prefix}{t}") for t in range(DT)]

            def assemble_x0(p, x0):
                # --- categorical gathers (dma_gather transpose) ---
                for f in range(N_CAT if "cat" in parts else 0):
                    trow = 960 + 64 * f
                    t, off = divmod(trow, 128)
                    direct = (f % 2 == 1) or f == 0
                    idx_ap = cidx[:, f * 128 + p * 64: f * 128 + p * 64 + 64]
                    if direct:
                        dst3 = x0[t][:].rearrange("q (a n) -> q a n", a=1)
                        nc.gpsimd.dma_gather(
                            out_ap=dst3, in_ap=cpad_d[f * CAT_VOCAB:(f + 1) * CAT_VOCAB, :],
                            idxs_ap=idx_ap, num_idxs=BC, num_idxs_reg=BC,
                            elem_size=128, transpose=True, single_packet=False)
                    else:
                        stg = stpool.tile([128, 1, BC], bf16, tag="cstg")
                        nc.gpsimd.dma_gather(
                            out_ap=stg[:], in_ap=cpad_d[f * CAT_VOCAB:(f + 1) * CAT_VOCAB, :],
                            idxs_ap=idx_ap, num_idxs=BC, num_idxs_reg=BC,
                            elem_size=128, transpose=True, single_packet=False)
                        nc.vector.tensor_tensor(x0[t][:], x0[t][:], stg[:, 0, :], ADD)
                # ones row for the bias fold (after f25's gather zeroed 64:128)
                nc.vector.memset(x0[20][64:65, :], 1.0)

                # --- numeric features: diag-expanded matmul ---
                for m in range(7 if "num" in parts else 0):
                    mw = 128 if m < 6 else 64
                    for ch in range(NCH):
                        ps = mmps.tile([128, 512], mybir.dt.float32, space="PSUM", tag="psacc")
                        nc.tensor.matmul(
                            ps[:mw, :], ndiag[:, m * 128: m * 128 + mw],
                            numT[:, p * BC + ch * 512: p * BC + (ch + 1) * 512],
                            start=True, stop=True)
                        if m < 6:
                            dst = x0[1 + m][:, ch * 512:(ch + 1) * 512]
                        else:
                            dst = x0[7][0:64, ch * 512:(ch + 1) * 512]
                        nc.scalar.activation(dst, ps[:mw, :], COPY)

                # --- user/item: indirect gather + PE transpose ---
                if "uit" not in parts:
                    return
                stu = stpool.tile([128, 8, 2, EMB], f32, tag="uit")
                for c in range(8):
                    pc = p * 8 + c
                    nc.gpsimd.indirect_dma_start(
                        out=stu[:, c, 0, :], out_offset=None, in_=uemb_d[:],
                        in_offset=bass.IndirectOffsetOnAxis(ap=uidx[:, pc:pc + 1], axis=0))
                    nc.gpsimd.indirect_dma_start(
                        out=stu[:, c, 1, :], out_offset=None, in_=iemb_d[:],
                        in_offset=bass.IndirectOffsetOnAxis(ap=iidx[:, pc:pc + 1], axis=0))
                for c in range(8):
                    pst = trps.tile([128, 128], f32, space="PSUM")
                    nc.tensor.transpose(pst[:], stu[:, c, :, :], ident[:])
                    nc.vector.tensor_copy(x0[0][:, c * 128:(c + 1) * 128], pst[:])

            def dense8(w_ap, xsrc8, evict, n_t=DT):
                """fp8 DoubleRow j-tile: n_t x 256-row contraction steps.
                One 2-bank [128, 1024] psum per j-tile (each matmul targets
                one bank half) so the evict runs once over the full BC."""
                wm = wpool.tile([128, DT * 256], f8, tag="wmain8")
                nc.sync.dma_start(wm[:, :n_t * 256], w_ap)
                ps = mmps.tile([128, NCH * 512], mybir.dt.float32, space="PSUM",
                               name="psacc", tag="psacc")
                for t in range(n_t):
                    st = wm[:, t * 256:(t + 1) * 256].rearrange("q (i n) -> q i n", i=2)
                    for ch in range(NCH):
                        nc.tensor.matmul(
                            ps[:, ch * 512:(ch + 1) * 512], st,
                            xsrc8[t][:, :, ch * 512:(ch + 1) * 512],
                            start=(t == 0), stop=(t == n_t - 1), perf_mode=DR)
                evict(ps)

            def cross_layer(i, x8_0, xsrc8, xdst8):
                for j in range(KT):
                    jw = 128 if j < 20 else 64
                    tj, ij = divmod(j, 2)

                    def evict(ps, j=j, jw=jw, tj=tj, ij=ij):
                        # elementwise x0 term read from the fp8 copy so the
                        # bf16 x0 staging is free for the next pass's gathers
                        tmp = tpool.tile([128, NCH * 512], bf16, tag="evt")
                        nc.vector.tensor_tensor(tmp[:jw, :], ps[:jw, :],
                                                x8_0[tj][:jw, ij, :], MULT)
                        nc.vector.tensor_tensor(xdst8[tj][:jw, ij, :], tmp[:jw, :],
                                                xsrc8[tj][:jw, ij, :], ADD)

                    dense8(Wc8_d[i, j], xsrc8, evict)

            def mlp_w0(x8_0, ha8):
                for j in range(MT):
                    def evict(ps, j=j):
                        nc.scalar.activation(
                            ha8[j // 2][:, j % 2, :], ps[:, :], RELU)
                    dense8(W08_d[j], x8_0, evict)

            def mlp_hidden(l, src8, dst8):
                for j in range(MT):
                    bias = bpool.tile([128, 1], f32, tag="bias")
                    nc.sync.dma_start(bias[:], bhT_d[j * 128:(j + 1) * 128, l:l + 1])

                    def evict(ps, j=j, bias=bias):
                        nc.scalar.activation(
                            dst8[j // 2][:, j % 2, :], ps[:, :],
                            RELU, bias=bias[:])

                    dense8(Wh8_d[l, j], src8, evict, n_t=4)

            def final_layer(p, xfin8, hfin8):
                # stationary n-dim padded to 16 (DoubleRow ISA: step%16==0);
                # psum rows 1:16 accumulate zeros, row 0 is the logit
                wf8 = wpool.tile([128, DT * 32], f8, tag="wf8")
                nc.sync.dma_start(wf8[:], Wf8_d[:])
                wfh8 = wpool.tile([128, 4 * 32], f8, tag="wfh8")
                nc.sync.dma_start(wfh8[:], Wfh8_d[:])
                for ch in range(NCH):
                    sl = slice(ch * 512, (ch + 1) * 512)
                    zps = mmps.tile([128, 512], mybir.dt.float32, space="PSUM", tag="psacc")
                    for t in range(DT):
                        st = wf8[:, t * 32:(t + 1) * 32].rearrange("q (i n) -> q i n", i=2)
                        nc.tensor.matmul(zps[0:16, :], st, xfin8[t][:, :, sl],
                                         start=(t == 0), stop=False, perf_mode=DR)
                    for t in range(4):
                        sth = wfh8[:, t * 32:(t + 1) * 32].rearrange("q (i n) -> q i n", i=2)
                        nc.tensor.matmul(zps[0:16, :], sth, hfin8[t][:, :, sl],
                                         start=False, stop=(t == 3),
                                         perf_mode=DR, skip_group_check=True)
                    zsb = zpool.tile([1, 512], f32, tag="zsb")
                    nc.scalar.activation(zsb[:], zps[0:1, :], SIGM)
                    nc.sync.dma_start(
                        out_d[0:1, p * BC + ch * 512: p * BC + (ch + 1) * 512], zsb[:])

            for pi, p in enumerate([pp for _ in range(repeats) for pp in range(N_PASS)]):
                x0 = [xpool.tile([128, BC], bf16, tag=f"x0_{t}", name=f"x0_{t}")
                      for t in range(KT)]
                assemble_x0(p, x0)
                # convert x0 -> fp8 double-k-tile layout
                # rotate the pass-initial fp8 family so pass p+1's convert
                # can run while pass p's cross layer 0 / W0 still read it
                x8_0 = alloc_x8(f"x8z{pi % 2}_")
                for ko in range(KT):
                    nc.vector.tensor_copy(x8_0[ko // 2][:, ko % 2, :], x0[ko][:, :])
                nc.vector.memset(x8_0[DT - 1][:, 1, :], 0.0)   # ko 21 zero pad
                fams = [alloc_x8("x8a_"), alloc_x8("x8b_")]
                for fam in fams:
                    # zero pad + bias-fold ones row in the last double tile
                    nc.vector.memset(fam[DT - 1][64:128, 0, :], 0.0)
                    nc.vector.memset(fam[DT - 1][64:65, 0, :], 1.0)
                    nc.vector.memset(fam[DT - 1][:, 1, :], 0.0)
                xsrc8 = x8_0
                for i in range(n_cross):
                    xdst8 = fams[i % 2]
                    cross_layer(i, x8_0, xsrc8, xdst8)
                    xsrc8 = xdst8
                xfin8 = xsrc8
                if with_mlp:
                    ha8 = [xpool.tile([128, 2, BC], f8, tag=f"h8a{t}", name=f"h8a{t}") for t in range(4)]
                    hb8 = [xpool.tile([128, 2, BC], f8, tag=f"h8b{t}", name=f"h8b{t}") for t in range(4)]
                    mlp_w0(x8_0, ha8)
                    hsrc8 = ha8
                    for l in range(N_MLP_HID):
                        hdst8 = hb8 if l % 2 == 0 else ha8
                        mlp_hidden(l, hsrc8, hdst8)
                        hsrc8 = hdst8
                    final_layer(p, xfin8, hsrc8)

    nc.compile()
    return nc


# needed at module level for the builder
import concourse.bass as bass  # noqa: E402


def _prep_core_inputs(core, user_input, item_input, numeric_feats, categorical_feats):
    r0 = core * B_CORE
    u = user_input[r0:r0 + B_CORE]
    it = item_input[r0:r0 + B_CORE]
    num = numeric_feats[r0:r0 + B_CORE]
    cat = categorical_feats[r0:r0 + B_CORE]

    u_idx = np.ascontiguousarray(u.reshape(16, 128).T).astype(np.int32)
    i_idx = np.ascontiguousarray(it.reshape(16, 128).T).astype(np.int32)

    c_idx = np.zeros((128, N_CAT * 128), np.int16)
    for f in range(N_CAT):
        for p in range(N_PASS):
            seg = cat[p * BC:(p + 1) * BC, f].astype(np.int16)
            blk = seg.reshape(BC // 16, 16).T          # wrap-A: idx i at [i%16, i//16]
            c_idx[:, f * 128 + p * 64: f * 128 + (p + 1) * 64] = np.tile(blk, (8, 1))

    import ml_dtypes
    numT = np.empty((N_NUM + 1, B_CORE), ml_dtypes.bfloat16)
    numT[:N_NUM] = num.T.astype(ml_dtypes.bfloat16)
    numT[N_NUM] = 1.0

    return {"u_idx": u_idx, "i_idx": i_idx, "c_idx": c_idx, "numT": numT}


def _fold_jblocks(W, bias, n_j):
    """[K, n_j*128 cols] + bias -> [n_j, 128, n_k*128] bf16-layout stationary
    blocks: out[j, q, t*128+n] = Wpad[t*128+q, j*128+n], bias at k-row K."""
    K, N = W.shape
    n_k = (K + 1 + 127) // 128
    Wpad = np.zeros((n_k * 128, n_j * 128), np.float32)
    Wpad[:K, :N] = W
    if bias is not None:
        Wpad[K, :N] = bias
    arr = Wpad.reshape(n_k, 128, n_j, 128)          # [t, q, j, n]
    return np.ascontiguousarray(arr.transpose(2, 1, 0, 3)).reshape(n_j, 128, n_k * 128)


def _fold_jblocks8(W, bias, n_j, n_t):
    """fp8 DoubleRow layout: out[j, q, ((t*2+i)*128+n)] = Wpad[t*256+i*128+q,
    j*128+n] with bias at k-row K, zero pad to n_t*256 rows."""
    K, N = W.shape
    Wpad = np.zeros((n_t * 256, n_j * 128), np.float32)
    Wpad[:K, :N] = W
    if bias is not None:
        Wpad[K, :N] = bias
    arr = Wpad.reshape(n_t, 2, 128, n_j, 128)       # [t, i, q, j, n]
    return np.ascontiguousarray(arr.transpose(3, 2, 0, 1, 4)).reshape(n_j, 128, n_t * 256)


def _prep_shared(num_W, num_b, user_emb, item_emb, cat_tables,
                 Wc, bc, W0, b0, Wh, bh, Wf, bf):
    import ml_dtypes
    bf16 = ml_dtypes.bfloat16
    f8 = ml_dtypes.float8_e4m3
    ndiag = np.zeros((N_NUM + 1, N_NUM * EMB), np.float32)
    for f in range(N_NUM):
        ndiag[f, f * EMB:(f + 1) * EMB] = num_W[f]
    ndiag[N_NUM] = num_b.reshape(-1)

    cat_pad = np.zeros((N_CAT * CAT_VOCAB, 128), bf16)
    ct = cat_tables.astype(bf16)
    for f in range(N_CAT):
        sl = slice(f * CAT_VOCAB, (f + 1) * CAT_VOCAB)
        if f % 2 == 0:   # destination rows 64:128 of the x^T tile
            cat_pad[sl, 64:128] = ct[f]
        else:            # destination rows 0:64
            cat_pad[sl, 0:64] = ct[f]

    WcP8 = np.stack([_fold_jblocks8(Wc[i], bc[i], KT, DT) for i in range(L_CROSS)])
    W0P8 = _fold_jblocks8(W0, b0, MT, DT)
    WhP8 = np.stack([_fold_jblocks8(Wh[l], None, MT, 4)
                     for l in range(N_MLP_HID)])

    kvec = np.zeros((DT * 256,), np.float32)
    kvec[:D] = Wf[:D, 0]
    kvec[D] = np.asarray(bf, np.float32).reshape(-1)[0]
    # [t, i, q] -> [q, t, i], n-dim zero-padded to 16 for the DoubleRow
    # step%16 ISA constraint (only n=0 carries the weight)
    WfP8x = np.zeros((128, DT, 2, 16), np.float32)
    WfP8x[:, :, :, 0] = kvec.reshape(DT, 2, 128).transpose(2, 0, 1)
    WfP8x = np.ascontiguousarray(WfP8x).reshape(128, DT * 32)
    WfPh8 = np.zeros((128, 4, 2, 16), np.float32)
    WfPh8[:, :, :, 0] = np.asarray(Wf, np.float32)[D:, 0].reshape(4, 2, 128).transpose(2, 0, 1)
    WfPh8 = np.ascontiguousarray(WfPh8).reshape(128, 128)

    return {
        "ndiag": ndiag.astype(bf16),
        "user_emb": np.ascontiguousarray(user_emb, np.float32),
        "item_emb": np.ascontiguousarray(item_emb, np.float32),
        "cat_pad": cat_pad,
        "WcP8": WcP8.astype(f8),
        "W0P8": W0P8.astype(f8),
        "WhP8": WhP8.astype(f8),
        "bhT": np.ascontiguousarray(np.asarray(bh, np.float32).T),
        "WfP8x": WfP8x.astype(f8),
        "WfPh8": WfPh8.astype(f8),
    }


def make_shared(user_emb, item_emb, cat_tables, num_W, num_b,
                Wc, bc, W0, b0, Wh, bh, Wf, bf, **_ignored):
    return _prep_shared(np.asarray(num_W, np.float32), np.asarray(num_b, np.float32),
                        np.asarray(user_emb), np.asarray(item_emb),
                        np.asarray(cat_tables, np.float32),
                        np.asarray(Wc, np.float32), np.asarray(bc, np.float32),
                        np.asarray(W0, np.float32), np.asarray(b0, np.float32),
                        np.asarray(Wh, np.float32), np.asarray(bh, np.float32),
                        np.asarray(Wf, np.float32), np.asarray(bf, np.float32))


def make_in_maps(user_input, item_input, numeric_feats, categorical_feats,
                 **_ignored):
    user_input = np.asarray(user_input).astype(np.int64)
    item_input = np.asarray(item_input).astype(np.int64)
    numeric_feats = np.asarray(numeric_feats, np.float32)
    categorical_feats = np.asarray(categorical_feats).astype(np.int64)
    return [
        _prep_core_inputs(core, user_input, item_input, numeric_feats,
                          categorical_feats)
        for core in range(CORES)
    ]


def _shared_hash(shared):
    h = hashlib.blake2b(digest_size=16)
    for k in sorted(shared):
        h.update(k.encode())
        h.update(np.ascontiguousarray(shared[k]).tobytes())
    return h.hexdigest()


def get_nc(shared=None, **flags):
    """Build (or fetch cached) compiled NC for the given constant set.

    With shared=None returns the most recently built NC (test harness
    convenience after a kernel() call)."""
    if shared is None:
        if not _CACHE:
            raise RuntimeError("no NC built yet; call kernel() first")
        return next(reversed(_CACHE.values()))
    key = (_shared_hash(shared), tuple(sorted(flags.items())))
    if key not in _CACHE:
        _CACHE[key] = _build_nc(shared, **flags)
    return _CACHE[key]


def kernel(**inputs) -> np.ndarray:
    from concourse.bass_utils import run_bass_kernel_spmd
    shared = make_shared(**inputs)
    nc = get_nc(shared)
    in_maps = make_in_maps(**inputs)
    res = run_bass_kernel_spmd(nc, in_maps, list(range(CORES)))
    out = np.concatenate([res.results[i]["out"][0] for i in range(CORES)])
    return out.reshape(B, 1).astype(np.float32)
